# revision 23
# baseline (speedup 1.0000x reference)
"""Trainium2 Bass kernel for nn_MultiHeadAttention (B=4, S=2048, D=512, H=8, DH=64).

Sharding: 8 cores = 4 batches x 2 query-halves. Each core computes full
attention for all 8 heads over its 1024 query rows (K/V projections are
duplicated within a batch pair). The output is a pure concatenation.

v2 datapath (natural-layout z + ScalarE-exclusive exp, all bf16):
  1. Host pre-transposes X to feature-major bf16 [D, S]; all loads are
     plain strided DMAs - no on-chip DMA transposes in the prologue.
  2. QKV projections in bf16 (optionally Q/K via fp8 DoubleRow with
     KERNEL_FP8QK=1; V path must stay bf16 for accuracy).
  3. Scores: bf16 dual-64 tile_position packing, S^T[k, q] per head pair.
  4. exp runs on ScalarE only (the binding engine, ~1.07us/[128,1024]
     tile); probs stay bf16 [k, q] per (pair-branch, kb).
  5. z accumulation in NATURAL layout: z[q, 130] += P_kb^T @ [VA|1|VB|1]
     per q-block: stationary = p-slice [128,128] (FWL fast weight load),
     moving = v_aug[kb] slice, N=65 per head -> ~2x fewer PE cycles than
     the old 65-partition z^T form. Denominators land at cols 64/129.
  6. Normalize = per-partition reciprocal columns + tensor_scalar muls
     (DVE); z then DMA-transposed (idle engines) to feature-major for
     the output projection.
  7. Output projection emitted transposed (out^T [D, SQ]) so the bias is
     per-partition; host un-transposes when assembling the result.

PSUM budget (8 banks): tags a, b = score tiles (2 banks each); tags
c0..c3 = one bank each, time-shared between projection pj chunks and the
per-pair zn accumulator (2 q-blocks per bank, chunk stride 256 floats).
"""

import os
import sys

import numpy as np

sys.path.insert(0, "/opt/trn_rl_repo")

import ml_dtypes
import concourse.bacc as bacc
import concourse.bass as bass
import concourse.mybir as mybir
import concourse.tile as tile
from concourse import bass_utils

F32 = mybir.dt.float32
BF16 = mybir.dt.bfloat16
FP8 = mybir.dt.float8e4
DR = mybir.MatmulPerfMode.DoubleRow

B, S, D, H, DH = 4, 2048, 512, 8, 64
SQ = S // 2          # query rows per core
NKB = S // 128       # 16 k-blocks
NFT = D // 128       # 4 feature tiles
NQB = SQ // 128      # 8 query blocks
N_CORES = 8

VW = 2 * DH + 2      # 130: [V_A | 1 | V_B | 1]
ZCH = 256            # zn chunk stride in floats (1KB); 2 chunks per bank

Exp = mybir.ActivationFunctionType.Exp

FP8QK = bool(int(os.environ.get("KERNEL_FP8QK", "0")))
WSCALE = 32.0 if FP8QK else 1.0
EXP_SCALE = 0.125 / (WSCALE * WSCALE)


def build_program(dbg=False):
    nc = bacc.Bacc("TRN2", target_bir_lowering=False, debug=False)
    dbg_out = {}
    if dbg:
        for nm, shp, dt in [("KT0", [128, S], BF16), ("QT0", [128, SQ], BF16),
                            ("VA0", [128, NFT * VW], BF16),
                            ("PA00", [128, SQ], BF16),
                            ("ZN0", [128, 2 * ZCH], F32),
                            ("ZT0", [128, SQ], BF16)]:
            dbg_out[nm] = nc.dram_tensor(nm, shp, dt, kind="ExternalOutput").ap()

    xdt = FP8 if FP8QK else BF16
    xqt = nc.dram_tensor("XQT", [D, SQ], xdt, kind="ExternalInput").ap()
    xkt = nc.dram_tensor("XKT", [D, S], xdt, kind="ExternalInput").ap()
    xvt = nc.dram_tensor("XVT", [D, S], BF16, kind="ExternalInput").ap()
    wq = nc.dram_tensor("WQP", [D, D], xdt, kind="ExternalInput").ap()
    wk = nc.dram_tensor("WKP", [D, D], xdt, kind="ExternalInput").ap()
    wv = nc.dram_tensor("WVP", [D, D], BF16, kind="ExternalInput").ap()
    wo = nc.dram_tensor("WOP", [D, D], BF16, kind="ExternalInput").ap()
    bq = nc.dram_tensor("bq", [D, 1], F32, kind="ExternalInput").ap()
    bk = nc.dram_tensor("bk", [D, 1], F32, kind="ExternalInput").ap()
    bv = nc.dram_tensor("bv", [1, D], F32, kind="ExternalInput").ap()
    bo = nc.dram_tensor("bo", [D, 1], F32, kind="ExternalInput").ap()
    out = nc.dram_tensor("OUTT", [D, SQ], F32, kind="ExternalOutput").ap()

    from contextlib import ExitStack

    with tile.TileContext(nc) as tc, ExitStack() as ctx:
        const = ctx.enter_context(tc.tile_pool(name="const", bufs=1))
        xt_pool = ctx.enter_context(tc.tile_pool(name="xt", bufs=1))
        w_pool = ctx.enter_context(tc.tile_pool(name="w", bufs=1))
        kt_pool = ctx.enter_context(tc.tile_pool(name="kt", bufs=1))
        qt_pool = ctx.enter_context(tc.tile_pool(name="qt", bufs=1))
        v_pool = ctx.enter_context(tc.tile_pool(name="v", bufs=1))
        p_pool = ctx.enter_context(tc.tile_pool(name="p", bufs=34))
        zt_pool = ctx.enter_context(tc.tile_pool(name="zt", bufs=1))
        nrm_pool = ctx.enter_context(tc.tile_pool(name="nrm", bufs=2))
        out_pool = ctx.enter_context(tc.tile_pool(name="outp", bufs=2))

        ps = ctx.enter_context(tc.tile_pool(name="ps", bufs=1, space="PSUM"))
        pj_ctr = [0]

        # pj chunks: [128, 512] f32 (1 bank), rotating tags c0..c3.
        def pj_tile(name):
            tag = f"c{pj_ctr[0] % 4}"
            pj_ctr[0] += 1
            return ps.tile([128, 512], F32, tag=tag, name=name,
                           padded_shape=[128, 512])

        # ---- SBUF allocations ------------------------------------------
        xq_s = xt_pool.tile([128, NFT * SQ], xdt, tag="xq", name="xq")
        xk_s = xt_pool.tile([128, NFT * S], xdt, tag="xk", name="xk")
        xv_s = xt_pool.tile([128, NFT * S], BF16, tag="xv", name="xv")
        wq_s = w_pool.tile([128, NFT * D], xdt, tag="wq", name="wq")
        wk_s = w_pool.tile([128, NFT * D], xdt, tag="wk", name="wk")
        wv_s = w_pool.tile([128, NFT * D], BF16, tag="wv", name="wv")
        wo_s = w_pool.tile([128, NFT * D], BF16, tag="wo", name="wo")

        # split chunk loads across the two HWDGE queues so transfers
        # parallelize over DMA engines
        dma_rr = [0]

        def dma_chunked(sbuf_tile, dram, lo=0, hi=None):
            ncols = dram.shape[1]
            if hi is None:
                hi = ncols
            for g in range(NFT):
                eng = (nc.sync, nc.scalar)[dma_rr[0] % 2]
                dma_rr[0] += 1
                eng.dma_start(
                    sbuf_tile[:, g * ncols + lo:g * ncols + hi],
                    dram[128 * g:128 * (g + 1), lo:hi],
                )

        def x3(t):
            return t[:].rearrange("p (g c) -> p g c", g=NFT)

        # ---- DMA loads, ordered by first use ----
        # xk is split by column halves: the first 8 score slots only need
        # k_t[0][:, 0:1024], so K-proj ft0/sc0 can start sooner.
        dma_chunked(wk_s, wk)
        dma_chunked(xk_s, xkt, 0, S // 2)
        bq_all = const.tile([128, NFT], F32, tag="bqa")
        nc.sync.dma_start(
            bq_all[:].rearrange("p (g o) -> p g o", g=NFT),
            bq.rearrange("(g p) o -> p g o", p=128),
        )
        bk_all = const.tile([128, NFT], F32, tag="bka")
        nc.sync.dma_start(
            bk_all[:].rearrange("p (g o) -> p g o", g=NFT),
            bk.rearrange("(g p) o -> p g o", p=128),
        )
        bo_all = const.tile([128, NFT], F32, tag="boa")
        nc.sync.dma_start(
            bo_all[:].rearrange("p (g o) -> p g o", g=NFT),
            bo.rearrange("(g p) o -> p g o", p=128),
        )
        bv_row = const.tile([1, D], F32, tag="bvr")
        nc.sync.dma_start(bv_row[:], bv[:])
        bv_bc = const.tile([128, D], F32, tag="bvb")
        nc.gpsimd.partition_broadcast(bv_bc[:], bv_row[:], channels=128)

        dma_chunked(wq_s, wq)
        dma_chunked(xq_s, xqt)
        dma_chunked(xk_s, xkt, S // 2, S)
        dma_chunked(wv_s, wv)
        dma_chunked(xv_s, xvt)
        dma_chunked(wo_s, wo)

        k_t = [kt_pool.tile([128, S], BF16, tag=f"kt{ft}", name=f"kt{ft}")
               for ft in range(NFT)]
        q_t = [qt_pool.tile([128, SQ], BF16, tag=f"qt{ft}", name=f"qt{ft}")
               for ft in range(NFT)]
        bq_t = [bq_all[:, ft:ft + 1] for ft in range(NFT)]
        bk_t = [bk_all[:, ft:ft + 1] for ft in range(NFT)]

        # Q/K projection: one pj chunk [128, 512] per N-half.
        def proj_chunk(w_s, x_s, dst, bias, ft, sc, h2s=(0, 1)):
            for h2 in h2s:
                lo = 1024 * sc + 512 * h2
                pj = pj_tile(f"pj{ft}{sc}{h2}")
                if FP8QK:
                    for kp in range(2):
                        nc.tensor.matmul(
                            pj[:],
                            x3(w_s)[:, 2 * kp:2 * kp + 2,
                                    128 * ft:128 * (ft + 1)],
                            x3(x_s)[:, 2 * kp:2 * kp + 2, lo:lo + 512],
                            start=(kp == 0), stop=(kp == 1),
                            perf_mode=DR,
                        )
                else:
                    for mc in range(NFT):
                        nc.tensor.matmul(
                            pj[:],
                            x3(w_s)[:, mc, 128 * ft:128 * (ft + 1)],
                            x3(x_s)[:, mc, lo:lo + 512],
                            start=(mc == 0), stop=(mc == NFT - 1),
                        )
                nc.vector.tensor_scalar_add(
                    dst[:, lo:lo + 512], pj[:], bias)

        def proj_k_chunk(ft, sc, h2s=(0, 1)):
            proj_chunk(wk_s, xk_s, k_t[ft], bk_t[ft], ft, sc, h2s)

        def proj_q_chunk(ft, h2s=(0, 1)):
            proj_chunk(wq_s, xq_s, q_t[ft], bq_t[ft], ft, 0, h2s)

        # six ~1us sub-chunks, spread over three slots (two per slot)
        def proj_kq_part(ft, part):
            if part == 0:
                proj_k_chunk(ft, 0)
            elif part == 1:
                proj_q_chunk(ft)
            else:
                proj_k_chunk(ft, 1)

        # warm the ScalarE Exp table before the critical path
        warm = nrm_pool.tile([1, 8], F32, tag="warm")
        nc.gpsimd.memset(warm[:], 0.0)
        warm2 = nrm_pool.tile([1, 8], F32, tag="warm2")
        nc.scalar.activation(warm2[:], warm[:], Exp, scale=EXP_SCALE)

        # ---- slot-scheduled emission ----------------------------------
        proj_k_chunk(0, 0)
        proj_q_chunk(0)

        # v_aug[kb]: all 4 head pairs side by side, each [V_A |1| V_B |1].
        v_aug = [v_pool.tile([128, NFT * VW], BF16, tag=f"v{kb}",
                             name=f"v{kb}")
                 for kb in range(NKB)]

        def v_group(kb):
            va4 = (v_aug[kb][:]
                   .rearrange("p (pr c) -> p pr c", c=VW)
                   .rearrange("p pr (h c) -> p pr h c", c=DH + 1))
            nc.gpsimd.memset(va4[:, :, :, DH:DH + 1], 1.0)
            pj = pj_tile(f"pjv{kb}")
            for mc in range(NFT):
                nc.tensor.matmul(
                    pj[:],
                    x3(xv_s)[:, mc, 128 * kb:128 * (kb + 1)],
                    x3(wv_s)[:, mc, :],
                    start=(mc == 0), stop=(mc == NFT - 1),
                )
            nc.vector.tensor_add(
                va4[:, :, :, 0:DH],
                pj[:].rearrange("p (pr h c) -> p pr h c", pr=NFT, h=2),
                bv_bc[:].rearrange("p (pr h c) -> p pr h c", pr=NFT, h=2),
            )

        zt_nat = [zt_pool.tile([128, SQ], BF16, tag=f"zn{p}", name=f"ztn{p}")
                  for p in range(NFT)]
        z_tt = [zt_pool.tile([128, SQ], BF16, tag=f"zt{p}", name=f"ztt{p}")
                for p in range(NFT)]
        p_slabs = {}
        z_acc = {}

        def s_exp(pair, kb):
            pA = p_pool.tile([128, SQ], BF16, tag="p", name=f"pA{pair}_{kb}")
            pB = p_pool.tile([128, SQ], BF16, tag="p", name=f"pB{pair}_{kb}")
            sA = ps.tile([128, SQ], F32, tag="a", name=f"sA{pair}_{kb}",
                         padded_shape=[128, SQ])
            sB = ps.tile([128, SQ], F32, tag="b", name=f"sB{pair}_{kb}",
                         padded_shape=[128, SQ])
            for qc in range(SQ // 512):
                qs = slice(512 * qc, 512 * (qc + 1))
                nc.tensor.matmul(
                    sA[:, qs],
                    k_t[pair][0:DH, 128 * kb:128 * (kb + 1)],
                    q_t[pair][0:DH, qs],
                    start=True, stop=True,
                    tile_position=(0, 0),
                )
            nc.scalar.activation(pA[:], sA[:], Exp, scale=EXP_SCALE)
            for qc in range(SQ // 512):
                qs = slice(512 * qc, 512 * (qc + 1))
                nc.tensor.matmul(
                    sB[:, qs],
                    k_t[pair][DH:128, 128 * kb:128 * (kb + 1)],
                    q_t[pair][DH:128, qs],
                    start=True, stop=True,
                    tile_position=(64, 0),
                )
            nc.scalar.activation(pB[:], sB[:], Exp, scale=EXP_SCALE)
            if dbg and pair == 0 and kb == 0:
                nc.sync.dma_start(dbg_out["PA00"][:], pA[:])
            p_slabs[(pair, kb)] = (pA, pB)

        def z_alloc(pair):
            z_acc[pair] = [
                ps.tile([128, 2 * ZCH], F32, tag=f"c{t}", name=f"zn{pair}_{t}",
                        padded_shape=[128, 2 * ZCH])
                for t in range(4)
            ]
            # full-bank memset: transfers byte ownership from the pj tiles
            # that previously lived in these banks (clean WAR edges)
            for t in range(4):
                nc.vector.memset(z_acc[pair][t][:], 0.0)

        # z natural: per q-block, zn[q, base:base+130] += p-slice^T @ v_aug.
        # start=True clears has_written for the WHOLE bank, so only the
        # first chunk written to each bank may use it; the bank-mates at
        # kb==0 rely on cleared bits -> overwrite semantics.
        def z_group(pair, kb):
            zn = z_acc[pair]
            pA, pB = p_slabs.pop((pair, kb))
            vA = v_aug[kb][:, VW * pair:VW * pair + VW]
            for qb in range(NQB):
                t, base = qb // 2, ZCH * (qb % 2)
                nc.tensor.matmul(
                    zn[t][:, base:base + DH + 1],
                    pA[:, 128 * qb:128 * (qb + 1)],
                    vA[:, 0:DH + 1],
                    start=(kb == 0 and qb % 2 == 0),
                    stop=(kb == NKB - 1),
                    skip_group_check=True,
                )
                nc.tensor.matmul(
                    zn[t][:, base + DH + 1:base + VW],
                    pB[:, 128 * qb:128 * (qb + 1)],
                    vA[:, DH + 1:VW],
                    start=False, stop=(kb == NKB - 1),
                    skip_group_check=True,
                )

        # normalize: denominators are cols 64 / 129 of each 130-col chunk
        def norm(pair):
            zn = z_acc.pop(pair)
            if dbg and pair == 0:
                zdump = out_pool.tile([128, 2 * ZCH], F32, tag="zdump")
                nc.vector.tensor_copy(zdump[:], zn[0][:])
                nc.sync.dma_start(dbg_out["ZN0"][:], zdump[:])
            den = nrm_pool.tile([128, 16], F32, tag=f"den{pair % 2}")
            for t in range(4):
                nc.vector.tensor_copy(
                    den[:, 4 * t:4 * t + 4].rearrange(
                        "p (j h) -> p j h", h=2),
                    zn[t][:].rearrange("p (j c) -> p j c", c=ZCH)
                    [:, :, DH:2 * DH + 2:DH + 1],
                )
            rec = nrm_pool.tile([128, 16], F32, tag=f"rec{pair % 2}")
            nc.vector.reciprocal_approx_fast(rec[:], den[:])
            # per q-block: normalize muls, then immediately transpose that
            # block to feature-major (DMA dispatch ~1.2us each: sync for
            # pairs 0-2, sync+scalar alternating for the last pair where
            # ScalarE has finished its exps)
            for qb in range(NQB):
                t, base = qb // 2, ZCH * (qb % 2)
                for h in range(2):
                    nc.vector.tensor_scalar_mul(
                        zt_nat[pair][:, 128 * qb + 64 * h:
                                     128 * qb + 64 * (h + 1)],
                        zn[t][:, base + (DH + 1) * h:
                              base + (DH + 1) * h + DH],
                        rec[:, 2 * qb + h:2 * qb + h + 1],
                    )
                eng = nc.scalar if (pair == NFT - 1 and qb % 2) else nc.sync
                eng.dma_start(
                    z_tt[pair][:, 128 * qb:128 * (qb + 1)],
                    zt_nat[pair][:, 128 * qb:128 * (qb + 1)],
                    transpose=True,
                )

        # slot schedule: filler work (z-groups, projections, v-groups) is
        # spread evenly so each slot's PE work stays under the ~2.1us
        # ScalarE exp budget and the score matmuls dispatch promptly.
        # pair-0 fillers: 8 proj sub-chunks (h2-granular) + 16 v-groups.
        feeder = []
        for ft, sc in ((0, 1), (1, 0), (None, None), (1, 1)):
            for h2 in range(2):
                if ft is None:
                    feeder.append((lambda h: lambda: proj_q_chunk(1, (h,)))(h2))
                else:
                    feeder.append((lambda f, s, h: lambda:
                                   proj_k_chunk(f, s, (h,)))(ft, sc, h2))
        feeder += [(lambda k: (lambda: v_group(k)))(kb) for kb in range(NKB)]
        fi = [0]

        def feed(n):
            for _ in range(n):
                if fi[0] < len(feeder):
                    feeder[fi[0]]()
                    fi[0] += 1

        # Gap slots after each norm(): the next pj/memset writes to the
        # c-tags must wait for norm's DVE readers, and a stalled PE
        # instruction at the queue head would block the score matmuls.
        for pair in range(NFT):
            for kb in range(NKB):
                s_exp(pair, kb)
                if pair == 0:
                    feed(2 if kb < 8 else 1)
                    if kb == NKB - 1:
                        feed(len(feeder))
                elif pair == 1:
                    if kb == 0:
                        z_alloc(0)
                    if kb < 8:
                        z_group(0, 2 * kb)
                        z_group(0, 2 * kb + 1)
                    elif kb == 8:
                        norm(0)
                    elif kb in (10, 11, 12):
                        proj_kq_part(2, kb - 10)
                    elif kb == 13:
                        z_alloc(1)
                    if kb >= 13:
                        z_group(1, kb - 13)
                elif pair == 2:
                    if kb < 6:
                        z_group(1, 3 + 2 * kb)
                        z_group(1, 3 + 2 * kb + 1)
                    elif kb == 6:
                        z_group(1, 15)
                        norm(1)
                    elif kb in (8, 9, 10):
                        proj_kq_part(3, kb - 8)
                    elif kb == 11:
                        z_alloc(2)
                    if kb >= 11:
                        z_group(2, 2 * (kb - 11))
                        z_group(2, 2 * (kb - 11) + 1)
                else:
                    if kb < 3:
                        z_group(2, 10 + 2 * kb)
                        z_group(2, 10 + 2 * kb + 1)
                    elif kb == 3:
                        norm(2)
                    if kb == 5:
                        z_alloc(3)
                    if 5 <= kb <= 10:
                        z_group(3, kb - 5)
                    elif kb >= 11:
                        z_group(3, 2 * kb - 16)
                        z_group(3, 2 * kb - 15)
            if pair == NFT - 1:
                norm(3)

        if dbg:
            nc.sync.dma_start(dbg_out["KT0"][:], k_t[0][:])
            nc.sync.dma_start(dbg_out["QT0"][:], q_t[0][:])
            nc.sync.dma_start(dbg_out["VA0"][:], v_aug[0][:])
            nc.sync.dma_start(dbg_out["ZT0"][:], z_tt[0][:])

        # ---- output projection (transposed: out^T[fo, q]) ----
        for fo in range(NFT):
            po = ps.tile([128, SQ], F32, tag="a" if fo % 2 == 0 else "b",
                         name=f"po{fo}", padded_shape=[128, SQ])
            for qc in range(SQ // 512):
                qs = slice(512 * qc, 512 * (qc + 1))
                for p4 in range(NFT):
                    nc.tensor.matmul(
                        po[:, qs],
                        x3(wo_s)[:, p4, 128 * fo:128 * (fo + 1)],
                        z_tt[p4][:, qs],
                        start=(p4 == 0),
                        stop=(p4 == NFT - 1),
                    )
            ot = out_pool.tile([128, SQ], F32, tag="ot")
            nc.vector.tensor_scalar_add(ot[:], po[:], bo_all[:, fo:fo + 1])
            nc.sync.dma_start(out[128 * fo:128 * (fo + 1), :], ot[:])

    nc.compile()
    return nc


_NC = None
LAST_RESULTS = None


def _get_nc():
    global _NC
    if _NC is None:
        _NC = build_program(dbg=bool(int(os.environ.get("KERNEL_DEBUG", "0"))))
    return _NC


def _bf(x):
    return np.ascontiguousarray(np.asarray(x, np.float32).astype(
        ml_dtypes.bfloat16))


def _fp8(x):
    return np.ascontiguousarray(
        np.clip(np.asarray(x, np.float32), -240.0, 240.0).astype(
            ml_dtypes.float8_e4m3))


def make_in_maps(Q, K, V, Wq, bq, Wk, bk, Wv, bv, Wo, bo):
    Qf = np.asarray(Q, np.float32)
    Kf = np.asarray(K, np.float32)
    Vf = np.asarray(V, np.float32)
    cast_qk = _fp8 if FP8QK else _bf
    shared = {
        "WQP": cast_qk(np.asarray(Wq, np.float32) * WSCALE),
        "WKP": cast_qk(np.asarray(Wk, np.float32) * WSCALE),
        "WVP": _bf(Wv),
        "WOP": _bf(Wo),
        "bq": np.ascontiguousarray(
            np.asarray(bq, np.float32).reshape(D, 1) * WSCALE),
        "bk": np.ascontiguousarray(
            np.asarray(bk, np.float32).reshape(D, 1) * WSCALE),
        "bv": np.ascontiguousarray(np.asarray(bv, np.float32).reshape(1, D)),
        "bo": np.ascontiguousarray(np.asarray(bo, np.float32).reshape(D, 1)),
    }
    in_maps = []
    for c in range(N_CORES):
        b, qh = c // 2, c % 2
        in_maps.append({
            "XQT": cast_qk(Qf[b, SQ * qh:SQ * (qh + 1)].T),
            "XKT": cast_qk(Kf[b].T),
            "XVT": _bf(Vf[b].T),
            **shared,
        })
    return in_maps


def kernel(Q, K, V, Wq, bq, Wk, bk, Wv, bv, Wo, bo):
    global LAST_RESULTS
    nc = _get_nc()
    in_maps = make_in_maps(Q, K, V, Wq, bq, Wk, bk, Wv, bv, Wo, bo)
    trace = bool(int(os.environ.get("KERNEL_TRACE", "0")))
    res = bass_utils.run_bass_kernel_spmd(
        nc, in_maps, core_ids=list(range(N_CORES)), trace=trace,
    )
    LAST_RESULTS = res
    out = np.empty((B, S, D), dtype=np.float32)
    for c in range(N_CORES):
        b, qh = c // 2, c % 2
        out[b, SQ * qh:SQ * (qh + 1)] = res.results[c]["OUTT"].T
    return out
